# revision 4
# baseline (speedup 1.0000x reference)
"""Trainium2 kernel for nn_Discriminator_26895085208120.

The reference circuit applies only single-qubit RX gates to |0...0> and
measures per-wire Pauli-Z. RX gates on the same wire compose by angle
addition (RX(a)RX(b) = RX(a+b)), gates on different wires act on disjoint
tensor factors, so the state stays a product state
    |psi> = prod_w [cos(phi_w/2), -i sin(phi_w/2)],  phi_w = x_w + theta_w
and <Z_w> = cos^2(phi_w/2) - sin^2(phi_w/2) = cos(x_w + theta_w).

The kernel therefore computes out[b, w] = cos(x[b, w] + thetas[w]) on
device: batch is sharded 4 rows per core across 8 cores (pure data
parallel), x arrives transposed [20 qubits, 4 batch] with qubits on SBUF
partitions, thetas [20, 1] replicated per core as the per-partition
activation bias, and a single ScalarEngine pass evaluates
sin(x + (theta + pi/2)).
"""

import math

import numpy as np

import concourse.bass as bass
import concourse.mybir as mybir
from concourse.bass_utils import run_bass_kernel_spmd

N_QUBITS = 20
BATCH = 32
N_CORES = 8
B_SHARD = BATCH // N_CORES  # 4 batch rows per core

_NC_CACHE = None


def build_nc() -> bass.Bass:
    nc = bass.Bass()
    x_d = nc.dram_tensor(
        "xT", [N_QUBITS, B_SHARD], mybir.dt.float32, kind="ExternalInput"
    )
    th_d = nc.dram_tensor(
        "th", [N_QUBITS, 1], mybir.dt.float32, kind="ExternalInput"
    )
    out_d = nc.dram_tensor(
        "out", [N_QUBITS, B_SHARD], mybir.dt.float32, kind="ExternalOutput"
    )

    # z = x + theta + pi/2 can land outside [-pi, pi], where the ACT Sin
    # table is garbage (verified on HW: exact inside, O(1) error past ~4.5).
    # Range-reduce with the f32 round-to-nearest magic constant 1.5*2^23:
    # k = round(z/2pi), z' = z - k*2pi in [-pi, pi], then sin(z').
    MAGIC = 12582912.0  # 1.5 * 2**23
    INV_2PI = 1.0 / (2.0 * math.pi)
    TWO_PI = 2.0 * math.pi

    with (
        nc.sbuf_tensor("x_t", [N_QUBITS, B_SHARD], mybir.dt.float32) as x_t,
        nc.sbuf_tensor("th_t", [N_QUBITS, 1], mybir.dt.float32) as th_t,
        nc.sbuf_tensor("z_t", [N_QUBITS, B_SHARD], mybir.dt.float32) as z_t,
        nc.sbuf_tensor("t_t", [N_QUBITS, B_SHARD], mybir.dt.float32) as t_t,
        nc.sbuf_tensor("k_t", [N_QUBITS, B_SHARD], mybir.dt.float32) as k_t,
        nc.sbuf_tensor("zr_t", [N_QUBITS, B_SHARD], mybir.dt.float32) as zr_t,
        nc.sbuf_tensor("o_t", [N_QUBITS, B_SHARD], mybir.dt.float32) as o_t,
        nc.semaphore("dma_sem") as dma_sem,
        nc.semaphore("dve_sem") as dve_sem,
        nc.semaphore("act_sem") as act_sem,
        nc.Block() as block,
    ):

        @block.sync
        def _(sync):
            sync.dma_start(out=x_t[:], in_=x_d[:]).then_inc(dma_sem, 16)
            sync.dma_start(out=th_t[:], in_=th_d[:]).then_inc(dma_sem, 16)
            sync.wait_ge(act_sem, 1)
            sync.dma_start(out=out_d[:], in_=o_t[:]).then_inc(dma_sem, 16)
            sync.wait_ge(dma_sem, 48)

        @block.vector
        def _(vector):
            # Chained same-engine ops need explicit sems: without them the
            # next op reads its input tile before the previous write lands
            # (verified on HW — stale/zero reads).
            vector.wait_ge(dma_sem, 32)
            # z = (x + theta) + pi/2
            vector.tensor_scalar(
                z_t[:],
                x_t[:],
                th_t[:, 0:1],
                math.pi / 2,
                mybir.AluOpType.add,
                mybir.AluOpType.add,
            ).then_inc(dve_sem, 1)
            vector.wait_ge(dve_sem, 1)
            # t = z/(2pi) + MAGIC  (forces round-to-nearest-integer in f32)
            vector.tensor_scalar(
                t_t[:],
                z_t[:],
                INV_2PI,
                MAGIC,
                mybir.AluOpType.mult,
                mybir.AluOpType.add,
            ).then_inc(dve_sem, 1)
            vector.wait_ge(dve_sem, 2)
            # k2pi = (t - MAGIC) * 2pi
            vector.tensor_scalar(
                k_t[:],
                t_t[:],
                MAGIC,
                TWO_PI,
                mybir.AluOpType.subtract,
                mybir.AluOpType.mult,
            ).then_inc(dve_sem, 1)
            vector.wait_ge(dve_sem, 3)
            # z' = z - k2pi  in [-pi, pi]
            vector.tensor_tensor(
                zr_t[:], z_t[:], k_t[:], mybir.AluOpType.subtract
            ).then_inc(dve_sem, 1)

        @block.scalar
        def _(scalar):
            scalar.wait_ge(dve_sem, 4)
            scalar.activation(
                o_t[:],
                zr_t[:],
                mybir.ActivationFunctionType.Sin,
                bias=0.0,
                scale=1.0,
            ).then_inc(act_sem, 1)

    return nc


def _make_in_maps(x: np.ndarray, thetas: np.ndarray) -> list[dict[str, np.ndarray]]:
    th = np.ascontiguousarray(thetas.reshape(N_QUBITS, 1), dtype=np.float32)
    in_maps = []
    for c in range(N_CORES):
        shard = np.ascontiguousarray(
            x[c * B_SHARD : (c + 1) * B_SHARD, :].T, dtype=np.float32
        )  # [N_QUBITS, B_SHARD]
        in_maps.append({"xT": shard, "th": th})
    return in_maps


def _gather(results: list[dict[str, np.ndarray]]) -> np.ndarray:
    return np.concatenate(
        [np.asarray(r["out"]).T for r in results], axis=0
    ).astype(np.float32)  # [BATCH, N_QUBITS]


def kernel(x, thetas, n_qubits) -> np.ndarray:
    global _NC_CACHE
    x = np.asarray(x, dtype=np.float32)
    thetas = np.asarray(thetas, dtype=np.float32)
    assert int(n_qubits) == N_QUBITS and x.shape == (BATCH, N_QUBITS)
    if _NC_CACHE is None:
        _NC_CACHE = build_nc()
    res = run_bass_kernel_spmd(
        _NC_CACHE, _make_in_maps(x, thetas), list(range(N_CORES))
    )
    return _gather(res.results)


def kernel_profiled(x, thetas, n_qubits):
    """Like kernel() but with NTFF tracing; returns (output, exec_time_ns)."""
    x = np.asarray(x, dtype=np.float32)
    thetas = np.asarray(thetas, dtype=np.float32)
    assert int(n_qubits) == N_QUBITS
    nc = build_nc()
    res = run_bass_kernel_spmd(
        nc, _make_in_maps(x, thetas), list(range(N_CORES)), trace=True
    )
    return _gather(res.results), res.exec_time_ns


# revision 5
# speedup vs baseline: 1.2503x; 1.2503x over previous
"""Trainium2 kernel for nn_Discriminator_26895085208120.

The reference circuit applies only single-qubit RX gates to |0...0> and
measures per-wire Pauli-Z. RX gates on the same wire compose by angle
addition (RX(a)RX(b) = RX(a+b)), gates on different wires act on disjoint
tensor factors, so the state stays a product state
    |psi> = prod_w [cos(phi_w/2), -i sin(phi_w/2)],  phi_w = x_w + theta_w
and <Z_w> = cos^2(phi_w/2) - sin^2(phi_w/2) = cos(x_w + theta_w).

The kernel therefore computes out[b, w] = cos(x[b, w] + thetas[w]) on
device: batch is sharded 4 rows per core across 8 cores (pure data
parallel), with qubits on SBUF partitions. Per core, one packed [20, 6]
DMA brings x^T (cols 0-3), theta (col 4) and a zero bias column (col 5);
the DVE computes z' = range-reduce(x + theta + pi/2) and the ACT engine
evaluates sin(z') (the HW Sin table is only valid on [-pi, pi] —
verified: exact inside, O(1) garbage beyond ~4.5).

Perf notes (measured on HW):
- A dummy Sin activation issued before any waits pulls the ~2.6us
  ACT_TABLE_LOAD+DRAIN off the critical path (overlaps the input DMA).
- Bass's init-time const-AP barrier and the Block-exit all-engine
  barrier cost ~8us combined; both are safe to suppress here (nothing
  reads the const-AP pool, and the Sync engine's final dma_sem wait
  already guarantees the output DMA completed before its stream ends).
- Chained same-engine DVE ops need explicit semaphore hops; without
  them the next op reads stale SBUF (verified on HW).
"""

import math

import numpy as np

import concourse.bass as bass
import concourse.mybir as mybir
from concourse.bass_utils import run_bass_kernel_spmd

N_QUBITS = 20
BATCH = 32
N_CORES = 8
B_SHARD = BATCH // N_CORES  # 4 batch rows per core

# packed input columns: [x0 x1 x2 x3 theta zero]
_XCOLS = B_SHARD
_PACKW = B_SHARD + 2

_NC_CACHE = None


class _FastBass(bass.Bass):
    """Bass with the init-time and Block-exit all-engine barriers removed."""

    def all_engine_barrier(self, *, sem_only: bool = False):
        return None


def build_nc() -> bass.Bass:
    nc = _FastBass()
    in_d = nc.dram_tensor(
        "inp", [N_QUBITS, _PACKW], mybir.dt.float32, kind="ExternalInput"
    )
    out_d = nc.dram_tensor(
        "out", [N_QUBITS, B_SHARD], mybir.dt.float32, kind="ExternalOutput"
    )

    # k = round(z/2pi) via the f32 round-to-nearest magic constant, then
    # z' = z - k*2pi in [-pi, pi].
    MAGIC = 12582912.0  # 1.5 * 2**23
    INV_2PI = 1.0 / (2.0 * math.pi)
    TWO_PI = 2.0 * math.pi

    with (
        nc.sbuf_tensor("in_t", [N_QUBITS, _PACKW], mybir.dt.float32) as in_t,
        nc.sbuf_tensor("z_t", [N_QUBITS, B_SHARD], mybir.dt.float32) as z_t,
        nc.sbuf_tensor("t_t", [N_QUBITS, B_SHARD], mybir.dt.float32) as t_t,
        nc.sbuf_tensor("k_t", [N_QUBITS, B_SHARD], mybir.dt.float32) as k_t,
        nc.sbuf_tensor("zr_t", [N_QUBITS, B_SHARD], mybir.dt.float32) as zr_t,
        nc.sbuf_tensor("o_t", [N_QUBITS, B_SHARD], mybir.dt.float32) as o_t,
        nc.sbuf_tensor("warm_t", [1, 1], mybir.dt.float32) as warm_t,
        nc.semaphore("dma_sem") as dma_sem,
        nc.semaphore("dve_sem") as dve_sem,
        nc.semaphore("act_sem") as act_sem,
        nc.Block(no_gpsimd_drain=True) as block,
    ):

        @block.sync
        def _(sync):
            sync.dma_start(out=in_t[:], in_=in_d[:]).then_inc(dma_sem, 16)
            sync.wait_ge(act_sem, 2)
            sync.dma_start(out=out_d[:], in_=o_t[:]).then_inc(dma_sem, 16)
            sync.wait_ge(dma_sem, 32)

        @block.vector
        def _(vector):
            vector.wait_ge(dma_sem, 16)
            # z = (x + theta) + pi/2
            vector.tensor_scalar(
                z_t[:],
                in_t[:, 0:_XCOLS],
                in_t[:, _XCOLS : _XCOLS + 1],
                math.pi / 2,
                mybir.AluOpType.add,
                mybir.AluOpType.add,
            ).then_inc(dve_sem, 1)
            vector.wait_ge(dve_sem, 1)
            # t = z/(2pi) + MAGIC
            vector.tensor_scalar(
                t_t[:],
                z_t[:],
                INV_2PI,
                MAGIC,
                mybir.AluOpType.mult,
                mybir.AluOpType.add,
            ).then_inc(dve_sem, 1)
            vector.wait_ge(dve_sem, 2)
            # k2pi = (t - MAGIC) * 2pi
            vector.tensor_scalar(
                k_t[:],
                t_t[:],
                MAGIC,
                TWO_PI,
                mybir.AluOpType.subtract,
                mybir.AluOpType.mult,
            ).then_inc(dve_sem, 1)
            vector.wait_ge(dve_sem, 3)
            # z' = z - k2pi  in [-pi, pi]
            vector.tensor_tensor(
                zr_t[:], z_t[:], k_t[:], mybir.AluOpType.subtract
            ).then_inc(dve_sem, 1)

        @block.scalar
        def _(scalar):
            # Dummy Sin on scratch: forces the ACT_TABLE_LOAD for the Sin
            # set here, overlapping the input DMA instead of serializing
            # after the DVE chain.
            scalar.activation(
                warm_t[:],
                warm_t[:],
                mybir.ActivationFunctionType.Sin,
                bias=warm_t[0:1, 0:1],
                scale=0.0,
            ).then_inc(act_sem, 1)
            scalar.wait_ge(dve_sem, 4)
            scalar.activation(
                o_t[:],
                zr_t[:],
                mybir.ActivationFunctionType.Sin,
                bias=in_t[:, _XCOLS + 1 : _XCOLS + 2],
                scale=1.0,
            ).then_inc(act_sem, 1)

    return nc


def _make_in_maps(x: np.ndarray, thetas: np.ndarray) -> list[dict[str, np.ndarray]]:
    in_maps = []
    for c in range(N_CORES):
        packed = np.zeros((N_QUBITS, _PACKW), dtype=np.float32)
        packed[:, 0:_XCOLS] = x[c * B_SHARD : (c + 1) * B_SHARD, :].T
        packed[:, _XCOLS] = thetas
        in_maps.append({"inp": packed})
    return in_maps


def _gather(results: list[dict[str, np.ndarray]]) -> np.ndarray:
    return np.concatenate(
        [np.asarray(r["out"]).T for r in results], axis=0
    ).astype(np.float32)  # [BATCH, N_QUBITS]


def kernel(x, thetas, n_qubits) -> np.ndarray:
    global _NC_CACHE
    x = np.asarray(x, dtype=np.float32)
    thetas = np.asarray(thetas, dtype=np.float32)
    assert int(n_qubits) == N_QUBITS and x.shape == (BATCH, N_QUBITS)
    if _NC_CACHE is None:
        _NC_CACHE = build_nc()
    res = run_bass_kernel_spmd(
        _NC_CACHE, _make_in_maps(x, thetas), list(range(N_CORES))
    )
    return _gather(res.results)


def kernel_profiled(x, thetas, n_qubits):
    """Like kernel() but with NTFF tracing; returns (output, exec_time_ns)."""
    x = np.asarray(x, dtype=np.float32)
    thetas = np.asarray(thetas, dtype=np.float32)
    assert int(n_qubits) == N_QUBITS
    nc = build_nc()
    res = run_bass_kernel_spmd(
        nc, _make_in_maps(x, thetas), list(range(N_CORES)), trace=True
    )
    return _gather(res.results), res.exec_time_ns


# revision 7
# speedup vs baseline: 1.4087x; 1.1267x over previous
"""Trainium2 kernel for nn_Discriminator_26895085208120.

The reference circuit applies only single-qubit RX gates to |0...0> and
measures per-wire Pauli-Z. RX gates on the same wire compose by angle
addition (RX(a)RX(b) = RX(a+b)), gates on different wires act on disjoint
tensor factors, so the state stays a product state
    |psi> = prod_w [cos(phi_w/2), -i sin(phi_w/2)],  phi_w = x_w + theta_w
and <Z_w> = cos^2(phi_w/2) - sin^2(phi_w/2) = cos(x_w + theta_w).

The kernel therefore computes out[b, w] = cos(x[b, w] + thetas[w]) on
device: batch is sharded 4 rows per core across 8 cores (pure data
parallel), with qubits on SBUF partitions. Per core, one packed [20, 6]
DMA brings x^T (cols 0-3), theta (col 4) and a zero bias column (col 5);
the DVE computes z' = range-reduce(x + theta + pi/2) and the ACT engine
evaluates sin(z') (the HW Sin table is only valid on [-pi, pi] —
verified: exact inside, O(1) garbage beyond ~4.5).

Perf notes (measured on HW):
- A dummy Sin activation issued before any waits pulls the ~2.6us
  ACT_TABLE_LOAD+DRAIN off the critical path (overlaps the input DMA).
- Bass's init-time const-AP barrier and the Block-exit all-engine
  barrier cost ~8us combined; both are safe to suppress here (nothing
  reads the const-AP pool, and the Sync engine's final dma_sem wait
  already guarantees the output DMA completed before its stream ends).
- Chained same-engine DVE ops need explicit semaphore hops; without
  them the next op reads stale SBUF (verified on HW).
"""

import math

import numpy as np

import concourse.bass as bass
import concourse.mybir as mybir
from concourse.bass_utils import run_bass_kernel_spmd

N_QUBITS = 20
BATCH = 32
N_CORES = 8
B_SHARD = BATCH // N_CORES  # 4 batch rows per core

# packed input columns: [x0 x1 x2 x3 theta zero]
_XCOLS = B_SHARD
_PACKW = B_SHARD + 2

_NC_CACHE = None


class _FastBass(bass.Bass):
    """Bass with the init-time and Block-exit all-engine barriers removed."""

    def all_engine_barrier(self, *, sem_only: bool = False):
        return None


def build_nc() -> bass.Bass:
    nc = _FastBass(monotonic_sem_count=0)
    in_d = nc.dram_tensor(
        "inp", [N_QUBITS, _PACKW], mybir.dt.float32, kind="ExternalInput"
    )
    out_d = nc.dram_tensor(
        "out", [N_QUBITS, B_SHARD], mybir.dt.float32, kind="ExternalOutput"
    )

    # k = round(z/2pi) via the f32 round-to-nearest magic constant, then
    # z' = z - k*2pi in [-pi, pi].
    MAGIC = 12582912.0  # 1.5 * 2**23
    INV_2PI = 1.0 / (2.0 * math.pi)
    TWO_PI = 2.0 * math.pi

    with (
        nc.sbuf_tensor("in_t", [N_QUBITS, _PACKW], mybir.dt.float32) as in_t,
        nc.sbuf_tensor("z_t", [N_QUBITS, B_SHARD], mybir.dt.float32) as z_t,
        nc.sbuf_tensor("t_t", [N_QUBITS, B_SHARD], mybir.dt.float32) as t_t,
        nc.sbuf_tensor("k_t", [N_QUBITS, B_SHARD], mybir.dt.float32) as k_t,
        nc.sbuf_tensor("zr_t", [N_QUBITS, B_SHARD], mybir.dt.float32) as zr_t,
        nc.sbuf_tensor("o_t", [N_QUBITS, B_SHARD], mybir.dt.float32) as o_t,
        nc.sbuf_tensor("warm_t", [1, 1], mybir.dt.float32) as warm_t,
        nc.semaphore("dma_sem") as dma_sem,
        nc.semaphore("dve_sem") as dve_sem,
        nc.semaphore("act_sem") as act_sem,
        nc.Block(no_gpsimd_drain=True) as block,
    ):

        @block.sync
        def _(sync):
            sync.dma_start(out=in_t[:], in_=in_d[:]).then_inc(dma_sem, 16)
            sync.wait_ge(act_sem, 2)
            sync.dma_start(out=out_d[:], in_=o_t[:]).then_inc(dma_sem, 16)
            sync.wait_ge(dma_sem, 32)

        @block.vector
        def _(vector):
            vector.wait_ge(dma_sem, 16)
            # z = (x + theta) + pi/2
            vector.tensor_scalar(
                z_t[:],
                in_t[:, 0:_XCOLS],
                in_t[:, _XCOLS : _XCOLS + 1],
                math.pi / 2,
                mybir.AluOpType.add,
                mybir.AluOpType.add,
            ).then_inc(dve_sem, 1)
            vector.wait_ge(dve_sem, 1)
            # t = z/(2pi) + MAGIC
            vector.tensor_scalar(
                t_t[:],
                z_t[:],
                INV_2PI,
                MAGIC,
                mybir.AluOpType.mult,
                mybir.AluOpType.add,
            ).then_inc(dve_sem, 1)
            vector.wait_ge(dve_sem, 2)
            # k2pi = (t - MAGIC) * 2pi
            vector.tensor_scalar(
                k_t[:],
                t_t[:],
                MAGIC,
                TWO_PI,
                mybir.AluOpType.subtract,
                mybir.AluOpType.mult,
            ).then_inc(dve_sem, 1)
            vector.wait_ge(dve_sem, 3)
            # z' = z - k2pi  in [-pi, pi]
            vector.tensor_tensor(
                zr_t[:], z_t[:], k_t[:], mybir.AluOpType.subtract
            ).then_inc(dve_sem, 1)

        @block.scalar
        def _(scalar):
            # Dummy Sin on scratch: forces the ACT_TABLE_LOAD for the Sin
            # set here, overlapping the input DMA instead of serializing
            # after the DVE chain.
            scalar.activation(
                warm_t[:],
                warm_t[:],
                mybir.ActivationFunctionType.Sin,
                bias=warm_t[0:1, 0:1],
                scale=0.0,
            ).then_inc(act_sem, 1)
            scalar.wait_ge(dve_sem, 4)
            scalar.activation(
                o_t[:],
                zr_t[:],
                mybir.ActivationFunctionType.Sin,
                bias=in_t[:, _XCOLS + 1 : _XCOLS + 2],
                scale=1.0,
            ).then_inc(act_sem, 1)

    # The PE engine and the Pool engine (only const-AP memsets, which
    # nothing reads) contribute no work; dropping their instructions lets
    # walrus emit fewer engine queues, shortening the NRT postamble
    # rendezvous by ~1.6us.
    drop = {mybir.EngineType.PE, mybir.EngineType.Pool}
    for bb in nc.m.functions[0].blocks:
        bb.instructions[:] = [i for i in bb.instructions if i.engine not in drop]

    return nc


def _make_in_maps(x: np.ndarray, thetas: np.ndarray) -> list[dict[str, np.ndarray]]:
    in_maps = []
    for c in range(N_CORES):
        packed = np.zeros((N_QUBITS, _PACKW), dtype=np.float32)
        packed[:, 0:_XCOLS] = x[c * B_SHARD : (c + 1) * B_SHARD, :].T
        packed[:, _XCOLS] = thetas
        in_maps.append({"inp": packed})
    return in_maps


def _gather(results: list[dict[str, np.ndarray]]) -> np.ndarray:
    return np.concatenate(
        [np.asarray(r["out"]).T for r in results], axis=0
    ).astype(np.float32)  # [BATCH, N_QUBITS]


def kernel(x, thetas, n_qubits) -> np.ndarray:
    global _NC_CACHE
    x = np.asarray(x, dtype=np.float32)
    thetas = np.asarray(thetas, dtype=np.float32)
    assert int(n_qubits) == N_QUBITS and x.shape == (BATCH, N_QUBITS)
    if _NC_CACHE is None:
        _NC_CACHE = build_nc()
    res = run_bass_kernel_spmd(
        _NC_CACHE, _make_in_maps(x, thetas), list(range(N_CORES))
    )
    return _gather(res.results)


def kernel_profiled(x, thetas, n_qubits):
    """Like kernel() but with NTFF tracing; returns (output, exec_time_ns)."""
    x = np.asarray(x, dtype=np.float32)
    thetas = np.asarray(thetas, dtype=np.float32)
    assert int(n_qubits) == N_QUBITS
    nc = build_nc()
    res = run_bass_kernel_spmd(
        nc, _make_in_maps(x, thetas), list(range(N_CORES)), trace=True
    )
    return _gather(res.results), res.exec_time_ns


# revision 11
# speedup vs baseline: 1.4532x; 1.0316x over previous
"""Trainium2 kernel for nn_Discriminator_26895085208120.

The reference circuit applies only single-qubit RX gates to |0...0> and
measures per-wire Pauli-Z. RX gates on the same wire compose by angle
addition (RX(a)RX(b) = RX(a+b)), gates on different wires act on disjoint
tensor factors, so the state stays a product state
    |psi> = prod_w [cos(phi_w/2), -i sin(phi_w/2)],  phi_w = x_w + theta_w
and <Z_w> = cos^2(phi_w/2) - sin^2(phi_w/2) = cos(x_w + theta_w).

The kernel therefore computes out[b, w] = cos(x[b, w] + thetas[w]) on
device: batch is sharded 4 rows per core across 8 cores (pure data
parallel), with qubits on SBUF partitions. Per core, one packed [20, 6]
DMA brings x^T (cols 0-3), theta (col 4) and a zero bias column (col 5);
the DVE computes z' = range-reduce(x + theta + pi/2) and the ACT engine
evaluates sin(z') (the HW Sin table is only valid on [-pi, pi] —
verified: exact inside, O(1) garbage beyond ~4.5).

Perf notes (measured on HW):
- A dummy Sin activation issued before any waits pulls the ~2.6us
  ACT_TABLE_LOAD+DRAIN off the critical path (overlaps the input DMA).
- Bass's init-time const-AP barrier and the Block-exit all-engine
  barrier cost ~8us combined; both are safe to suppress here (nothing
  reads the const-AP pool, and the Sync engine's final dma_sem wait
  already guarantees the output DMA completed before its stream ends).
- Chained same-engine DVE ops need explicit semaphore hops; without
  them the next op reads stale SBUF (verified on HW).
"""

import math

import numpy as np

import concourse.bass as bass
import concourse.mybir as mybir
from concourse.bass_utils import run_bass_kernel_spmd

N_QUBITS = 20
BATCH = 32
N_CORES = 8
B_SHARD = BATCH // N_CORES  # 4 batch rows per core

# packed input columns: [x0 x1 x2 x3 theta zero]
_XCOLS = B_SHARD
_PACKW = B_SHARD + 2

_NC_CACHE = None


class _FastBass(bass.Bass):
    """Bass with the init-time and Block-exit all-engine barriers removed."""

    def all_engine_barrier(self, *, sem_only: bool = False):
        return None


def build_nc() -> bass.Bass:
    nc = _FastBass(monotonic_sem_count=0)
    in_d = nc.dram_tensor(
        "inp", [N_QUBITS, _PACKW], mybir.dt.float32, kind="ExternalInput"
    )
    out_d = nc.dram_tensor(
        "out", [N_QUBITS, B_SHARD], mybir.dt.float32, kind="ExternalOutput"
    )

    # k = round(z/2pi) via the f32 round-to-nearest magic constant, then
    # z' = z - k*2pi in [-pi, pi].
    MAGIC = 12582912.0  # 1.5 * 2**23
    INV_2PI = 1.0 / (2.0 * math.pi)
    TWO_PI = 2.0 * math.pi

    with (
        nc.sbuf_tensor("in_t", [N_QUBITS, _PACKW], mybir.dt.float32) as in_t,
        nc.sbuf_tensor("z_t", [N_QUBITS, B_SHARD], mybir.dt.float32) as z_t,
        nc.sbuf_tensor("t_t", [N_QUBITS, B_SHARD], mybir.dt.float32) as t_t,
        nc.sbuf_tensor("k_t", [N_QUBITS, B_SHARD], mybir.dt.float32) as k_t,
        nc.sbuf_tensor("zr_t", [N_QUBITS, B_SHARD], mybir.dt.float32) as zr_t,
        nc.sbuf_tensor("o_t", [N_QUBITS, B_SHARD], mybir.dt.float32) as o_t,
        nc.sbuf_tensor("warm_t", [1, 1], mybir.dt.float32) as warm_t,
        nc.semaphore("dma_sem") as dma_sem,
        nc.semaphore("dve_sem") as dve_sem,
        nc.semaphore("act_sem") as act_sem,
        nc.Block(no_gpsimd_drain=True) as block,
    ):

        @block.sync
        def _(sync):
            sync.dma_start(out=in_t[:], in_=in_d[:]).then_inc(dma_sem, 16)
            sync.wait_ge(act_sem, 2)
            sync.dma_start(out=out_d[:], in_=o_t[:]).then_inc(dma_sem, 16)
            # Required: NEFF completion does not imply in-flight DMA
            # completion (verified: dropping this corrupts the output).
            sync.wait_ge(dma_sem, 32)

        @block.vector
        def _(vector):
            vector.wait_ge(dma_sem, 16)
            # z = (x + theta) + pi/2
            vector.tensor_scalar(
                z_t[:],
                in_t[:, 0:_XCOLS],
                in_t[:, _XCOLS : _XCOLS + 1],
                math.pi / 2,
                mybir.AluOpType.add,
                mybir.AluOpType.add,
            ).then_inc(dve_sem, 1)
            vector.wait_ge(dve_sem, 1)
            # t = z/(2pi) + MAGIC
            vector.tensor_scalar(
                t_t[:],
                z_t[:],
                INV_2PI,
                MAGIC,
                mybir.AluOpType.mult,
                mybir.AluOpType.add,
            ).then_inc(dve_sem, 1)
            vector.wait_ge(dve_sem, 2)
            # k2pi = (t - MAGIC) * 2pi
            vector.tensor_scalar(
                k_t[:],
                t_t[:],
                MAGIC,
                TWO_PI,
                mybir.AluOpType.subtract,
                mybir.AluOpType.mult,
            ).then_inc(dve_sem, 1)
            vector.wait_ge(dve_sem, 3)
            # z' = z - k2pi  in [-pi, pi]
            vector.tensor_tensor(
                zr_t[:], z_t[:], k_t[:], mybir.AluOpType.subtract
            ).then_inc(dve_sem, 1)

        @block.scalar
        def _(scalar):
            # Dummy Sin on scratch: forces the ACT_TABLE_LOAD for the Sin
            # set here, overlapping the input DMA instead of serializing
            # after the DVE chain. Its increment goes to act_sem only —
            # it must never satisfy a dve_sem wait (that exact bug caused
            # intermittent stale reads in an earlier merged-sem version).
            scalar.activation(
                warm_t[:],
                warm_t[:],
                mybir.ActivationFunctionType.Sin,
                bias=warm_t[0:1, 0:1],
                scale=0.0,
            ).then_inc(act_sem, 1)
            scalar.wait_ge(dve_sem, 4)
            scalar.activation(
                o_t[:],
                zr_t[:],
                mybir.ActivationFunctionType.Sin,
                bias=in_t[:, _XCOLS + 1 : _XCOLS + 2],
                scale=1.0,
            ).then_inc(act_sem, 1)

    # The PE engine and the Pool engine (only const-AP memsets, which
    # nothing reads) contribute no work; dropping their instructions lets
    # walrus emit fewer engine queues, shortening the NRT postamble
    # rendezvous by ~1.6us. (Dropping SP too — ACT-triggered DMAs — ran
    # ~0.8us faster still, but caused intermittent NRT_EXEC_UNIT_
    # UNRECOVERABLE device crashes, so SP keeps the DMAs.)
    drop = {mybir.EngineType.PE, mybir.EngineType.Pool}
    for bb in nc.m.functions[0].blocks:
        bb.instructions[:] = [i for i in bb.instructions if i.engine not in drop]

    return nc


def _make_in_maps(x: np.ndarray, thetas: np.ndarray) -> list[dict[str, np.ndarray]]:
    in_maps = []
    for c in range(N_CORES):
        packed = np.zeros((N_QUBITS, _PACKW), dtype=np.float32)
        packed[:, 0:_XCOLS] = x[c * B_SHARD : (c + 1) * B_SHARD, :].T
        packed[:, _XCOLS] = thetas
        in_maps.append({"inp": packed})
    return in_maps


def _gather(results: list[dict[str, np.ndarray]]) -> np.ndarray:
    return np.concatenate(
        [np.asarray(r["out"]).T for r in results], axis=0
    ).astype(np.float32)  # [BATCH, N_QUBITS]


def kernel(x, thetas, n_qubits) -> np.ndarray:
    global _NC_CACHE
    x = np.asarray(x, dtype=np.float32)
    thetas = np.asarray(thetas, dtype=np.float32)
    assert int(n_qubits) == N_QUBITS and x.shape == (BATCH, N_QUBITS)
    if _NC_CACHE is None:
        _NC_CACHE = build_nc()
    res = run_bass_kernel_spmd(
        _NC_CACHE, _make_in_maps(x, thetas), list(range(N_CORES))
    )
    return _gather(res.results)


def kernel_profiled(x, thetas, n_qubits):
    """Like kernel() but with NTFF tracing; returns (output, exec_time_ns)."""
    x = np.asarray(x, dtype=np.float32)
    thetas = np.asarray(thetas, dtype=np.float32)
    assert int(n_qubits) == N_QUBITS
    nc = build_nc()
    res = run_bass_kernel_spmd(
        nc, _make_in_maps(x, thetas), list(range(N_CORES)), trace=True
    )
    return _gather(res.results), res.exec_time_ns
